# revision 2
# baseline (speedup 1.0000x reference)
"""DialogueGCN forward as a Bass/Tile kernel on 8 TRN2 NeuronCores — v2.

Changes vs baseline: bf16 operands everywhere (PSUM accum stays fp32), all
inputs packed into 4 DMAs, scores reshaped [1,2304]->[48,48] by a direct
PSUM->SBUF DMA (no DRAM bounce, no single-partition copies), RGCN kept in
h^T orientation end-to-end (no transposes), and a parameterized sharding:
GROUP=1 replicates the full problem per core (no collective at all);
GROUP=g>1 shards H/G by g within groups of g cores and AllGathers h^T.
"""
import numpy as np
import ml_dtypes

L = 48
D = 256
H = 256
A = 128
NREL = 8
N_CORES = 8

GROUP = 1          # cores per shard-group; 1 = full replication, no collective
HS = H // GROUP    # rows of h^T this core produces
GS = H // GROUP    # rows of out^T this core produces
HSP = min(HS, 128)
GSP = min(GS, 128)
N_HCH = (HS + 127) // 128
N_GCH = (GS + 127) // 128

BF = ml_dtypes.bfloat16

# apack column offsets
_XT0, _XT1 = 0, 48
_WQ = 96
_WK = 352
_V = 608
APACK_COLS = 609
MPACK_COLS = 392

_compiled = None


def _emit_body(nc, mybir, pool, psum, dram, d, rep, collective):
    bf = mybir.dt.bfloat16
    f32 = mybir.dt.float32
    u = f"_{rep}"

    # ---- packed input DMAs on four queues ----
    apk = pool.tile([128, APACK_COLS], bf, name=f"apk{u}", tag="apk")
    mpk = pool.tile([128, MPACK_COLS], bf, name=f"mpk{u}", tag="mpk")
    rpk = pool.tile([128, 2, 9, HS], bf, name=f"rpk{u}", tag="rpk")
    gpk = pool.tile([128, 2, 2, GS], bf, name=f"gpk{u}", tag="gpk")
    nc.sync.dma_start(apk[:], d["apack"].ap())
    nc.scalar.dma_start(rpk[:], d["rpack"].ap().rearrange(
        "p (t r h) -> p t r h", t=2, r=9))
    nc.gpsimd.dma_start(mpk[:], d["mpack"].ap())
    nc.gpsimd.dma_start(gpk[:], d["gpack"].ap().rearrange(
        "p (t s g) -> p t s g", t=2, s=2))

    def xt(t):
        return apk[:, 48 * t:48 * t + 48]
    def wq(t):
        return apk[:, _WQ + 128 * t:_WQ + 128 * t + 128]
    def wk(t):
        return apk[:, _WK + 128 * t:_WK + 128 * t + 128]
    vv = apk[:, _V:_V + 1]
    maskt = mpk[0:L, 0:NREL * L].rearrange("i (r j) -> i r j", r=NREL)
    brg = mpk[:, 384:384 + 2 * N_HCH].bitcast(mybir.dt.float32)
    bgc = mpk[:, 388:388 + 2 * N_GCH].bitcast(mybir.dt.float32)

    # ---- attention: qT/kT in one PSUM bank ----
    qk_ps = psum.tile([128, 2, L], f32, name=f"qk_ps{u}", tag="P_qk")
    for t in range(2):
        nc.tensor.matmul(qk_ps[:, 0, :], wq(t), xt(t), start=(t == 0), stop=(t == 1))
    for t in range(2):
        nc.tensor.matmul(qk_ps[:, 1, :], wk(t), xt(t), start=(t == 0), stop=(t == 1))
    qkTs = pool.tile([128, 2, L], bf, name=f"qkTs{u}", tag="qkTs")
    nc.vector.tensor_copy(qkTs[:], qk_ps[:])

    # bigT[a, j, i] = kT[a, j] + qT[a, i]  (j-major!); tanh; then 48 tiny
    # matmuls lhsT=tanh[:, j, :] x rhs=v write scores directly as [48, 48]
    # PSUM columns -- no flat layout, no copies, no reshape DMA.
    NCH = 3
    CJ = L // NCH  # 16 key columns per chunk
    bigT = pool.tile([128, L, L], bf, name=f"bigT{u}", tag="bigT")
    tanhT = pool.tile([128, L * L], bf, name=f"tanhT{u}", tag="tanhT")
    scores_ps = psum.tile([L, L], f32, name=f"scores_ps{u}", tag="P_sc")
    for c in range(NCH):
        jsl = slice(c * CJ, (c + 1) * CJ)
        csl = slice(c * CJ * L, (c + 1) * CJ * L)
        nc.vector.tensor_tensor(
            bigT[:, jsl, :],
            qkTs[:, 1, jsl].rearrange("p (j o) -> p j o", o=1).broadcast_to([128, CJ, L]),
            qkTs[:, 0:1, :].broadcast_to([128, CJ, L]),
            op=mybir.AluOpType.add,
        )
        nc.scalar.activation(tanhT[:, csl],
                             bigT[:, jsl, :].rearrange("p j i -> p (j i)"),
                             mybir.ActivationFunctionType.Tanh)
        for j in range(c * CJ, (c + 1) * CJ):
            nc.tensor.matmul(scores_ps[:, j:j + 1],
                             tanhT[:, j * L:(j + 1) * L], vv,
                             start=True, stop=True)

    # softmax rows + per-relation masking (exp reads PSUM directly)
    expS = pool.tile([L, L], f32, name=f"expS{u}", tag="expS")
    rowsum = pool.tile([L, 1], f32, name=f"rowsum{u}", tag="rowsum")
    nc.scalar.activation(expS[:], scores_ps[:],
                         mybir.ActivationFunctionType.Exp,
                         accum_out=rowsum[:])
    recip = pool.tile([L, 1], f32, name=f"recip{u}", tag="recip")
    nc.vector.reciprocal(recip[:], rowsum[:])
    attnW = pool.tile([L, NREL, L], bf, name=f"attnW{u}", tag="attnW")
    NH = NREL // 2
    for a in range(2):
        rsl = slice(a * NH, (a + 1) * NH)
        nc.vector.scalar_tensor_tensor(
            attnW[:, rsl, :],
            expS[:].rearrange("i (o j) -> i o j", o=1).broadcast_to([L, NH, L]),
            recip[:],
            maskt[:, rsl, :],
            op0=mybir.AluOpType.mult,
            op1=mybir.AluOpType.mult,
        )

    # ---- RGCN: yall = x @ W_rel (all live relations), h^T orientation ----
    yall = pool.tile([L, NREL * HS], bf, name=f"yall{u}", tag="yall")
    ycols = NREL * HS
    ytags = ("P_y0", "P_y1")
    rview = rpk[:].rearrange("p t r h -> p t (r h)")
    nchunk = (ycols + 511) // 512
    for ci in range(nchunk):
        lo, hi = ci * 512, min((ci + 1) * 512, ycols)
        yp = psum.tile([L, hi - lo], f32, name=f"yp{u}_{ci}", tag=ytags[ci % 2])
        for t in range(2):
            nc.tensor.matmul(yp[:], xt(t), rview[:, t, lo:hi],
                             start=(t == 0), stop=(t == 1))
        if ci % 2 == 0:
            nc.vector.tensor_copy(yall[:, lo:hi], yp[:])
        else:
            nc.scalar.copy(yall[:, lo:hi], yp[:])

    aggT_ps = psum.tile([HSP, N_HCH, L], f32, name=f"aggT_ps{u}", tag="P_qk")
    for hc in range(N_HCH):
        hsl = slice(hc * 128, hc * 128 + HSP)
        for t in range(2):
            nc.tensor.matmul(aggT_ps[:, hc, :], rpk[:, t, 8, hsl], xt(t),
                             start=(t == 0), stop=False)
        for r in range(NREL):
            nc.tensor.matmul(aggT_ps[:, hc, :],
                             yall[:, r * HS + hc * 128:r * HS + hc * 128 + HSP],
                             attnW[:, r, :],
                             start=False, stop=(r == NREL - 1))
    hT = pool.tile([HSP, N_HCH, L], bf, name=f"hT{u}", tag="hT")
    for hc in range(N_HCH):
        nc.vector.tensor_scalar_add(hT[:, hc, :], aggT_ps[:, hc, :],
                                    brg[0:HSP, hc:hc + 1])

    # ---- assemble full h^T [2, 128, 48] ----
    if GROUP == 1:
        hfull = hT  # [128, 2, L] already the full thing
    else:
        ag_in = dram.tile([HS, L], bf, name=f"ag_in{u}", tag="ag_in")
        ag_out = dram.tile([H, L], bf, name=f"ag_out{u}", tag="ag_out")
        nc.sync.dma_start(ag_in[:], hT[:, 0, :] if N_HCH == 1 else hT[:])
        if collective:
            groups = [list(range(b, b + GROUP))
                      for b in range(0, N_CORES, GROUP)]
            nc.gpsimd.collective_compute(
                "AllGather",
                mybir.AluOpType.bypass,
                replica_groups=groups,
                ins=[ag_in.opt()],
                outs=[ag_out.opt()],
            )
        else:
            agw = ag_out[:].rearrange("(c p) f -> c p f", p=HS)
            for c in range(GROUP):
                nc.sync.dma_start(agw[c], ag_in[:])
        hfull = pool.tile([128, 2, L], bf, name=f"hfull{u}", tag="hfull")
        agv = ag_out[:].rearrange("(t p) f -> t p f", p=128)
        nc.sync.dma_start(hfull[:, 0, :], agv[0])
        nc.scalar.dma_start(hfull[:, 1, :], agv[1])

    # ---- GraphConv ----
    sT = pool.tile([128, 2, 1], bf, name=f"sT{u}", tag="sT")
    with nc.allow_low_precision(reason="48-elem sum; bf16 feeds matmul rhs"):
        for t in range(2):
            nc.vector.reduce_sum(sT[:, t, :], hfull[:, t, :],
                                 axis=mybir.AxisListType.X)
    nb_ps = psum.tile([GSP, N_GCH, 1], f32, name=f"nb_ps{u}", tag="P_y0")
    for gc_ in range(N_GCH):
        gsl = slice(gc_ * 128, gc_ * 128 + GSP)
        for t in range(2):
            nc.tensor.matmul(nb_ps[:, gc_, :], gpk[:, t, 1, gsl], sT[:, t, :],
                             start=(t == 0), stop=(t == 1))
    nbs = pool.tile([GSP, N_GCH, 1], f32, name=f"nbs{u}", tag="nbs")
    for gc_ in range(N_GCH):
        nc.vector.tensor_scalar_add(nbs[:, gc_, :], nb_ps[:, gc_, :],
                                    bgc[0:GSP, gc_:gc_ + 1])
    gc_ps = psum.tile([GSP, N_GCH, L], f32, name=f"gc_ps{u}", tag="P_sc")
    for gc_ in range(N_GCH):
        gsl = slice(gc_ * 128, gc_ * 128 + GSP)
        for t in range(2):
            nc.tensor.matmul(gc_ps[:, gc_, :], gpk[:, t, 0, gsl], hfull[:, t, :],
                             start=(t == 0), stop=(t == 1))
    outs = pool.tile([GSP, N_GCH, L], f32, name=f"outs{u}", tag="outs")
    for gc_ in range(N_GCH):
        nc.vector.tensor_scalar_add(outs[:, gc_, :], gc_ps[:, gc_, :],
                                    nbs[:, gc_, :])
    if GROUP == 1:
        nc.sync.dma_start(d["yout"].ap().rearrange("(g p) f -> p g f", p=128),
                          outs[:])
    else:
        nc.sync.dma_start(d["yout"].ap(), outs[:, 0, :])


def build_program(n_cores=N_CORES, collective=True, repeat=1):
    import concourse.bacc as bacc
    import concourse.mybir as mybir
    import concourse.tile as tile

    bf = mybir.dt.bfloat16
    nc = bacc.Bacc("TRN2", debug=False, num_devices=n_cores)
    d = {}
    d["apack"] = nc.dram_tensor("apack", [128, APACK_COLS], bf, kind="ExternalInput")
    d["mpack"] = nc.dram_tensor("mpack", [128, MPACK_COLS], bf, kind="ExternalInput")
    d["rpack"] = nc.dram_tensor("rpack", [128, 18 * HS], bf, kind="ExternalInput")
    d["gpack"] = nc.dram_tensor("gpack", [128, 4 * GS], bf, kind="ExternalInput")
    if GROUP == 1:
        d["yout"] = nc.dram_tensor("yout", [H, L], mybir.dt.float32,
                                   kind="ExternalOutput")
    else:
        d["yout"] = nc.dram_tensor("yout", [GS, L], mybir.dt.float32,
                                   kind="ExternalOutput")

    with tile.TileContext(nc) as tc:
        with (
            tc.tile_pool(name="sbuf", bufs=1) as pool,
            tc.tile_pool(name="psum", bufs=1, space="PSUM") as psum,
            tc.tile_pool(name="dram", bufs=1, space="DRAM") as dram,
        ):
            for rep in range(repeat):
                _emit_body(nc, mybir, pool, psum, dram, d, rep, collective)
    nc.compile()
    return nc


def _prepare_in_maps(global_features, speaker, Wq, Wk, v, W_rel, W_root, b_rgcn,
                     W_nbr, W_self, b_gcn):
    f32 = np.float32
    x = np.asarray(global_features, dtype=f32)
    sp = np.asarray(speaker).astype(np.int64)
    n = L

    ii, jj = np.meshgrid(np.arange(n), np.arange(n), indexing="ij")
    direction = (ii >= jj).astype(np.int64)
    et = 2 * (sp[ii] * n + sp[jj]) + direction
    rel_ids = np.unique(et)
    assert len(rel_ids) <= NREL
    masks = np.zeros((NREL, n, n), dtype=f32)
    rel_pad = np.full(NREL, rel_ids[0], dtype=np.int64)
    for s, rid in enumerate(rel_ids):
        masks[s] = (et == rid)
        rel_pad[s] = rid
    W_used = np.asarray(W_rel)[rel_pad].astype(f32)       # [8, 256, 256]

    xT = x.T                                              # [256, 48]
    Wq = np.asarray(Wq, f32)
    Wk = np.asarray(Wk, f32)
    apack = np.concatenate(
        [xT[0:128], xT[128:256], Wq[0:128], Wq[128:256], Wk[0:128], Wk[128:256],
         np.asarray(v, f32).reshape(128, 1)], axis=1).astype(BF)

    maskw = masks.transpose(1, 0, 2).reshape(L, NREL * L)  # [48, 384]
    b_rgcn = np.asarray(b_rgcn, f32)
    b_gcn = np.asarray(b_gcn, f32)
    W_root = np.asarray(W_root, f32)
    W_self = np.asarray(W_self, f32)
    W_nbr = np.asarray(W_nbr, f32)

    in_maps = []
    for c in range(N_CORES):
        cc = c % GROUP if GROUP > 1 else 0
        slH = slice(cc * HS, (cc + 1) * HS)
        mpack = np.zeros((128, MPACK_COLS), BF)
        mpack[0:L, 0:NREL * L] = maskw.astype(BF)
        brg_c = np.zeros((128, N_HCH), f32)
        bgc_c = np.zeros((128, N_GCH), f32)
        brg_c[0:HSP] = b_rgcn[slH].reshape(N_HCH, HSP).T
        bgc_c[0:GSP] = b_gcn[slH].reshape(N_GCH, GSP).T
        mpack[:, 384:384 + 2 * N_HCH] = brg_c.view(BF)
        mpack[:, 388:388 + 2 * N_GCH] = bgc_c.view(BF)

        wrel_c = W_used[:, :, slH].reshape(NREL, 2, 128, HS).transpose(2, 1, 0, 3)
        root_c = W_root[:, slH].reshape(2, 128, HS).transpose(1, 0, 2)
        rpack = np.concatenate(
            [wrel_c, root_c[:, :, None, :]], axis=2).reshape(128, 18 * HS).astype(BF)

        self_c = W_self[:, slH].reshape(2, 128, GS).transpose(1, 0, 2)
        nbr_c = W_nbr[:, slH].reshape(2, 128, GS).transpose(1, 0, 2)
        gpack = np.stack([self_c, nbr_c], axis=2).reshape(128, 4 * GS).astype(BF)

        in_maps.append({"apack": apack, "mpack": mpack,
                        "rpack": rpack, "gpack": gpack})
    return in_maps


def kernel(global_features, speaker, Wq, Wk, v, W_rel, W_root, b_rgcn,
           W_nbr, W_self, b_gcn):
    global _compiled
    from concourse.bass_utils import run_bass_kernel_spmd

    if _compiled is None:
        _compiled = build_program()
    nc = _compiled
    in_maps = _prepare_in_maps(global_features, speaker, Wq, Wk, v, W_rel,
                               W_root, b_rgcn, W_nbr, W_self, b_gcn)
    res = run_bass_kernel_spmd(nc, in_maps, core_ids=list(range(N_CORES)))
    if GROUP == 1:
        outT = res.results[0]["yout"]                      # [256, 48]
    else:
        outT = np.concatenate([res.results[c]["yout"] for c in range(GROUP)],
                              axis=0)
    return np.ascontiguousarray(outT.T.astype(np.float32))
